# revision 2
# baseline (speedup 1.0000x reference)
"""Inverse Haar DWT2 (pywt 'haar' idwt2 convention) on 8 Trainium2 cores.

Input  x: [16, 256, 128, 128] f32 — 4 stacked subbands (LL|LH|HL|HH) of 64
channels each.  Output: [16, 64, 256, 256] f32.

Sharding: batch dim (16) split across 8 cores, 2 batches per core.  The
transform is elementwise per (batch, channel) — no communication.

Per-core kernel (x_loc [2, 256, 128, 128] -> y_loc [2, 64, 256, 256]):
SBUF partition dim = (batch, channel) = 2*64 = 128; free dim = a chunk of
hc input rows * 128 cols (hc=8 steady state, 4/4 tail).  Per tile:
  - 4 DMAs (one per subband) load T f32 [128p, band*hc*128] on the SP
    HW-DGE ring, band order in SBUF = (LL, HL, LH, HH); DRAM-side inner
    runs are hc*512B contiguous
  - cast+prescale (ACT): Tb bf16 <- T * (0.5 for LL,LH | 1.0 for HL,HH).
    Halving LL,LH here folds the first Haar 0.5; bf16 intermediates make
    stage 1 eligible for the DVE 2x packed perf mode (needs all-2B dtypes
    and unit strides) and roundoff (~2^-9) is far inside the 2e-2 gate.
  - stage 1 (DVE, bf16, fully contiguous halves):
    UV[0:2fb] = Tb[0:2fb]+Tb[2fb:4fb] = (P|R),  P=(LL+LH)/2, R=HL+HH
    UV[2fb:4fb] = Tb[0:2fb]-Tb[2fb:4fb] = (Q|S), Q=(LL-LH)/2, S=HL-HH
  - stage 2 (DVE scalar_tensor_tensor, bf16 in / f32 out):
    out[2i+rr, 2j+s] = (R_or_S * +-0.5) + P_or_Q — the remaining Haar
    0.5 is folded into the stt scalar
  - 1 DMA stores OUT f32 on the ACT HW-DGE ring (separate ring from loads
    so a compute-gated store never head-of-line-blocks loads), emitted one
    iteration late so it also never blocks the next cast on the ACT queue
Engine budget per hc=8 tile: DVE ~6.5us, ACT ~4.8us, DMA ~10.2us
-> DMA-bound with comfortable DVE/ACT slack (the previous all-f32 version
ran DVE at 9us/tile, neck-and-neck with DMA, and pipeline jitter starved
the SDMA engines).  pin bufs=6 gives ~3 tiles of load-ahead so load issue
is never gated on DVE completion.

This container's walrus build supports only ONE semaphore wait per
instruction; Tile emits multi-wait instructions (incl. the final drain), so
after TileContext exit we redistribute extra waits onto single-wait NOPs
inserted before the instruction on the same engine.
"""

import numpy as np

import concourse.bass as bass
import concourse.mybir as mybir
from concourse.tile import TileContext
from concourse.bass_utils import run_bass_kernel_spmd

N_CORES = 8
B, C4, H, W = 16, 256, 128, 128
CH = C4 // 4          # 64 output channels
B_LOC = B // N_CORES  # 2 batches per core
HC = 8                # input rows per tile iteration
F32 = mybir.dt.float32
BF16 = mybir.dt.bfloat16

# SBUF band order (LL, HL, LH, HH): T[0:2fb]+T[2fb:4fb] = (LL+LH | HL+HH)
# = (2P | R) and the difference gives (2Q | S) with fully contiguous APs.
BAND_SRC = (0, 2, 1, 3)  # T band t <- DRAM band BAND_SRC[t]


def _split_multi_waits(nc):
    """Move extra semaphore waits onto single-wait NOPs placed immediately
    before the over-subscribed instruction (same engine, so per-engine
    program order is preserved)."""
    n_split = 0
    for f in nc.m.functions:
        for blk in f.blocks:
            il = blk.instructions
            new_list = []
            for inst in il:
                si = getattr(inst, "sync_info", None)
                ow = si.on_wait if si is not None else None
                if ow and len(ow) > 1:
                    extra = list(ow[:-1])
                    del ow[:-1]
                    for w in extra:
                        n_split += 1
                        new_list.append(
                            mybir.InstNoOp(
                                name=f"{inst.name}-waitsplit-{n_split}",
                                engine=inst.engine,
                                sync_info=mybir.SyncInfo(on_wait=[w], on_update=[]),
                            )
                        )
                new_list.append(inst)
            il[:] = new_list
    return n_split


def _build_kernel():
    nc = bass.Bass("TRN2")
    x = nc.dram_tensor("x", [B_LOC, C4, H, W], F32, kind="ExternalInput")
    y = nc.dram_tensor("y", [B_LOC, CH, 2 * H, 2 * W], F32, kind="ExternalOutput")

    # Short taper so the post-last-load drain is cheap without making the
    # tail tiles DVE-bound (hc=2 tiles were: DVE per-op overhead dominates).
    tiles = [(i * HC, HC) for i in range(15)] + [(120, 4), (124, 4)]
    assert sum(hc for _, hc in tiles) == H

    with TileContext(nc) as tc:
        with (
            tc.tile_pool(name="tin", bufs=6) as pin,
            tc.tile_pool(name="tbf", bufs=2) as pbf,
            tc.tile_pool(name="tuv", bufs=2) as puv,
            tc.tile_pool(name="tout", bufs=3) as pout,
        ):
            pending_store = None  # (OUT tile, h0, hc) deferred one iteration

            def flush_store():
                pOUT, ph0, phc = pending_store
                nc.scalar.dma_start(
                    out=y[:, :, 2 * ph0 : 2 * ph0 + 2 * phc, :]
                    .rearrange("b c h w -> c b (h w)"),
                    in_=pOUT[:],
                )

            for h0, hc in tiles:
                fb = hc * W  # free elems per band block
                # ---- load: T [p=(c,b)][band][i][w]  (SP HW-DGE ring)
                # partition p = c*2 + b so the DRAM AP's outermost dim has
                # count 64 (the HWDGE engine spray follows the outer source
                # dim; outer count 2 would use only 2 of 16 SDMA engines)
                T = pin.tile([128, 4 * fb], F32, tag="T")
                for tband in range(4):
                    sband = BAND_SRC[tband]
                    nc.sync.dma_start(
                        out=T[:, tband * fb : (tband + 1) * fb],
                        in_=x[:, sband * CH : (sband + 1) * CH, h0 : h0 + hc, :]
                        .rearrange("b c h w -> c b (h w)"),
                    )
                # ---- cast+prescale (ACT): Tb bf16 <- T * (0.5|1.0).
                # Bands {0,2} (LL,LH) get the first Haar 0.5; bands {1,3}
                # (HL,HH) are cast unscaled (their 0.5 rides the stage-2
                # stt scalar).  band = g*2 + b2 -> (g, b2, x) view, one ACT
                # op per b2 with 2 free dims.
                Tb = pbf.tile([128, 4 * fb], BF16, tag="Tb")
                Tg = T[:].rearrange("p (g b2 x) -> p g b2 x", g=2, b2=2)
                Tbg = Tb[:].rearrange("p (g b2 x) -> p g b2 x", g=2, b2=2)
                nc.scalar.mul(Tbg[:, :, 0], Tg[:, :, 0], 0.5)
                nc.scalar.mul(Tbg[:, :, 1], Tg[:, :, 1], 1.0)
                # ---- deferred store of the PREVIOUS tile.  Emitting it
                # after this iteration's casts gives the ACT queue slack
                # before a stage-2-gated store could head-of-line-block the
                # next cast.
                if pending_store is not None:
                    flush_store()
                # ---- stage 1 (DVE, bf16): butterfly over contiguous
                # halves.  1 free dim, unit stride, all-bf16 -> 2x packed
                # perf mode.
                UV = puv.tile([128, 4 * fb], BF16, tag="UV")  # [P|R|Q|S]
                nc.vector.tensor_add(
                    out=UV[:, : 2 * fb], in0=Tb[:, : 2 * fb], in1=Tb[:, 2 * fb :]
                )
                nc.vector.tensor_sub(
                    out=UV[:, 2 * fb :], in0=Tb[:, : 2 * fb], in1=Tb[:, 2 * fb :]
                )
                # ---- stage 2 (DVE): out[2i+rr, 2j+s] = P_or_Q + (-1)^s *
                # R_or_S / 2, computed as (R * +-0.5) + P with
                # scalar_tensor_tensor.  OUT free layout [i][rr][col],
                # col = 2j+s.  Keep every AP at <=2 free dims.
                OUT = pout.tile([128, 2 * hc * 2 * W], F32, tag="OUT")
                OUTv = OUT[:].rearrange(
                    "p (i r j s) -> p i r j s", i=hc, r=2, j=W, s=2
                )
                UVq = UV[:].rearrange("p (q i w) -> p q i w", q=4, i=hc)
                for rr in range(2):
                    P = UVq[:, 2 * rr]      # P (rr=0) or Q (rr=1), pre-halved
                    R = UVq[:, 2 * rr + 1]  # R (rr=0) or S (rr=1)
                    nc.vector.scalar_tensor_tensor(
                        out=OUTv[:, :, rr, :, 0], in0=R, scalar=0.5, in1=P,
                        op0=mybir.AluOpType.mult, op1=mybir.AluOpType.add,
                    )
                    nc.vector.scalar_tensor_tensor(
                        out=OUTv[:, :, rr, :, 1], in0=R, scalar=-0.5, in1=P,
                        op0=mybir.AluOpType.mult, op1=mybir.AluOpType.add,
                    )
                # ---- store (ACT HW-DGE ring, deferred one iteration so
                # stores never head-of-line-block loads or casts)
                pending_store = (OUT, h0, hc)
            # flush the final deferred store
            flush_store()

    _split_multi_waits(nc)
    return nc


_NC_CACHE = None


def _get_nc():
    global _NC_CACHE
    if _NC_CACHE is None:
        _NC_CACHE = _build_kernel()
    return _NC_CACHE


def run_sharded(x, trace=False, **kwargs):
    assert x.shape == (B, C4, H, W) and x.dtype == np.float32
    nc = _get_nc()
    in_maps = [
        {"x": np.ascontiguousarray(x[i * B_LOC : (i + 1) * B_LOC])}
        for i in range(N_CORES)
    ]
    res = run_bass_kernel_spmd(
        nc, in_maps, core_ids=list(range(N_CORES)), trace=trace, **kwargs
    )
    out = np.concatenate([r["y"] for r in res.results], axis=0)
    return out, res


def kernel(x):
    out, _ = run_sharded(np.asarray(x))
    return out
